# revision 69
# baseline (speedup 1.0000x reference)
"""Trainium2 Bass kernel for a 12-head attention block with post-softmax
additive per-head bias.

    qkv = x @ W_qkv                          x: [64, 196, 768]
    attn = softmax(q k^T / 8) + static_a     (bias added AFTER softmax)
    out = (attn @ v) @ W_proj + b_proj

Sharding: data-parallel over batch across 8 NeuronCores (8 batches each).
No collectives needed. Weights are replicated; x is passed transposed
([768, 1568] per core) so the contraction dim lands on SBUF partitions
without any on-chip transpose. Compute dtype bf16 (PE 1 cyc/row vs 4 for
f32), f32 PSUM accumulation.

Per-core dataflow, software-pipelined over the 8 local batches b:
  qkT(b)  = W_{q,k}^T @ x_b^T      (head-pair packed on partitions)
  v(b)    = x_b @ W_v              (65-stride layout with a ones column)
  S^T(b)  = k @ q^T  -> exp (ACT, scale=1/8, fused PSUM->SBUF) -> P^T
  AV(b)   = A_h @ v                (per-head bias term, ACT-copied to O)
  U'(b-1) = P @ [v|1]              (ones column gives softmax row sums)
  O(b-1)  = U * (1/r) + AV         (DVE)
  O^T(b-1) via PE transpose -> attn_outT
  out     = attn_out @ W_proj + b_proj  (bias via broadcast-DMA + DVE add)

The b-1 stages overlap ACT's exp(b), keeping the TensorEngine busy.
"""

import os
import sys

_TRN_REPO = "/opt/trn_rl_repo"
if _TRN_REPO not in sys.path:
    sys.path.insert(0, _TRN_REPO)

import numpy as np
import ml_dtypes

import concourse.bass as bass
import concourse.tile as tile
from concourse import bacc, mybir
from concourse.bass import MemorySpace
from concourse.bass_utils import run_bass_kernel_spmd
from concourse.masks import make_identity

BF16 = mybir.dt.bfloat16
F32 = mybir.dt.float32

N_CORES = 8
BATCH = 64
B = BATCH // N_CORES  # 8 local batches per core
H = 12
D = 64
N = 196
C = 768
T = B * N  # 1568 local tokens
KC = 6  # contraction chunks of 128 over C=768
SCALE = D ** -0.5  # 0.125

# token chunks of 128 over T (for the projection)
MCS = [(i * 128, min(128, T - i * 128)) for i in range((T + 127) // 128)]
# per-batch row chunks over N=196
NCH = [(0, 128), (128, 68)]

AluOp = mybir.AluOpType
ActFn = mybir.ActivationFunctionType


def _emit(nc: bass.Bass):
    # xt: per-batch partition-major x^T blocks: xt[b, p, kc*N+n] = x[b, n, kc*128+p]
    # at: partition-major A^T: at[mc, p, h*N+n] = A[h, n, mc*128+p]
    xt_d = nc.declare_dram_parameter("xt", [B, 128, KC * N], BF16, isOutput=False)
    wqkv_d = nc.declare_dram_parameter("wqkv", [C, 3 * C], BF16, isOutput=False)
    at_d = nc.declare_dram_parameter("at", [2, 128, H * N], BF16, isOutput=False)
    wproj_d = nc.declare_dram_parameter("wproj", [C, C], BF16, isOutput=False)
    bproj_d = nc.declare_dram_parameter("bproj", [1, C], BF16, isOutput=False)
    out_d = nc.declare_dram_parameter("out", [T, C], F32, isOutput=True)

    with tile.TileContext(nc) as tc:
        from contextlib import ExitStack

        with ExitStack() as stk:
            const = stk.enter_context(tc.tile_pool(name="const", bufs=1))
            wq = stk.enter_context(tc.tile_pool(name="wq", bufs=1))
            xtp = stk.enter_context(tc.tile_pool(name="xtp", bufs=4))
            qkp = stk.enter_context(tc.tile_pool(name="qkp", bufs=4))
            vbp = stk.enter_context(tc.tile_pool(name="vbp", bufs=4))
            obp = stk.enter_context(tc.tile_pool(name="obp", bufs=4))
            ptp = stk.enter_context(tc.tile_pool(name="ptp", bufs=6))
            stage = stk.enter_context(tc.tile_pool(name="stage", bufs=3))
            small = stk.enter_context(tc.tile_pool(name="small", bufs=3))
            outst = stk.enter_context(tc.tile_pool(name="outst", bufs=4))
            aotp = stk.enter_context(tc.tile_pool(name="aotp", bufs=1))

            # ---- constants (cheap; loaded early) ----
            ident = const.tile([128, 128], BF16)
            make_identity(nc, ident)
            zbias = const.tile([128, 1], F32)
            nc.vector.memset(zbias, 0.0)

            wqkv_sb = wq.tile([128, KC, 3 * C], BF16)
            at_sb = const.tile([128, 2, H * N], BF16)
            wproj_sb = const.tile([128, KC, C], BF16)
            bias_bc = const.tile([128, C], BF16)
            aot_sb = aotp.tile([128, KC, T], BF16)

            # per-batch rotating tiles, tracked across loop iterations
            xtb_t = {}
            qtb_t = {}
            ktb_t = {}
            vb_t = {}
            ob_t = {}
            pt_t = {}

            def emit_load_x(b):
                xtb = xtp.tile([128, KC, N], BF16, tag="xtb")
                xtb_t[b] = xtb
                nc.sync.dma_start(
                    out=xtb.rearrange("p k n -> p (k n)"), in_=xt_d[b]
                )

            def emit_qkT(b):
                xtb = xtb_t[b]
                qtb = qkp.tile([128, KC, N], BF16, tag="qtb")
                ktb = qkp.tile([128, KC, N], BF16, tag="ktb")
                qtb_t[b], ktb_t[b] = qtb, ktb
                # stage odd heads' q^T/k^T down to base partition 0: a 64-row
                # matmul at base partition 64 followed by one at base 0
                # crashes the hardware, so S^T always reads base-0 operands.
                # Each staging DMA issues as soon as its section's copies land.
                stq = stage.tile([64, KC, N], BF16, tag="stq")
                stk_ = stage.tile([64, KC, N], BF16, tag="stk")
                for dst, sec, stg in ((qtb, 0, stq), (ktb, C, stk_)):
                    for c in range(KC):
                        ps = psA.tile([128, 512], F32, tag="pA")
                        for kc in range(KC):
                            nc.tensor.matmul(
                                ps[:, 0:N],
                                lhsT=wqkv_sb[
                                    :, kc, sec + c * 128 : sec + (c + 1) * 128
                                ],
                                rhs=xtb[:, kc, :],
                                start=(kc == 0),
                                stop=(kc == KC - 1),
                            )
                        nc.vector.tensor_copy(dst[:, c, :], ps[:, 0:N])
                    nc.sync.dma_start(out=stg, in_=dst[64:128, :, :])
                return stq, stk_

            def emit_v(b):
                xtb = xtb_t[b]
                vb = vbp.tile([128, 2, H * 65], BF16, tag="vb")
                vb_t[b] = vb
                for mc, (moff, mlen) in enumerate(NCH):
                    vv = vb[0:mlen, mc, :].rearrange("p (h x) -> p h x", h=H)
                    nc.vector.memset(vv[:, :, 64:65], 1.0)
                    for ns in range(2):
                        ps = psA.tile([128, 512], F32, tag="pA")
                        for kc in range(KC):
                            nc.tensor.matmul(
                                ps[0:mlen, 0:384],
                                lhsT=xtb[:, kc, moff : moff + mlen],
                                rhs=wqkv_sb[
                                    :, kc, 2 * C + ns * 384 : 2 * C + (ns + 1) * 384
                                ],
                                start=(kc == 0),
                                stop=(kc == KC - 1),
                            )
                        nc.vector.tensor_copy(
                            vv[:, ns * 6 : (ns + 1) * 6, 0:64],
                            ps[0:mlen, 0:384].rearrange("p (h c) -> p h c", h=6),
                        )

            def emit_st_av(b, stq, stk_):
                # Interleave the A@v matmul blocks between the exp-paced S^T
                # PSUM groups: PE executes in order, so a stalled S^T group
                # would otherwise idle the array while ACT drains exp.
                qtb, ktb = qtb_t[b], ktb_t[b]
                vb = vb_t[b]
                ob = obp.tile([128, 2, C], BF16, tag="ob")
                ob_t[b] = ob

                def st_unit(mc, hg):
                    moff, mlen = NCH[mc]
                    pt = pt_t[(b, mc)]
                    ps = psB.tile([128, 1024], F32, tag="pB")
                    for hh in range(4):
                        h = hg * 4 + hh
                        off = (hh // 2) * 512 + (hh % 2) * 196
                        if h % 2 == 0:
                            lhsT = ktb[0:64, h // 2, moff : moff + mlen]
                            rhs = qtb[0:64, h // 2, :]
                        else:
                            lhsT = stk_[0:64, h // 2, moff : moff + mlen]
                            rhs = stq[0:64, h // 2, :]
                        nc.tensor.matmul(
                            ps[0:mlen, off : off + 196],
                            lhsT=lhsT,
                            rhs=rhs,
                            start=True,
                            stop=True,
                        )
                    src = ps.rearrange("p (k x) -> p k x", k=2)[
                        0:mlen, :, 0:392
                    ].rearrange("p k (h n) -> p k h n", h=2)
                    dst = pt[0:mlen, hg * 4 * N : (hg + 1) * 4 * N].rearrange(
                        "p (k h n) -> p k h n", k=2, h=2
                    )
                    nc.scalar.activation(
                        dst, src, ActFn.Exp, bias=zbias[0:mlen, :], scale=SCALE
                    )

                av_tile = {}

                def av_unit(nc_i, hblk):
                    noff, nlen = NCH[nc_i]
                    if nc_i not in av_tile:
                        av = psB.tile([128, 1024], F32, tag="pB")
                        av_tile[nc_i] = av
                    av = av_tile[nc_i]
                    for h in range(hblk * 3, hblk * 3 + 3):
                        aoff = (h // 8) * 512 + (h % 8) * 64
                        for mc, (moff, mlen) in enumerate(NCH):
                            nc.tensor.matmul(
                                av[0:nlen, aoff : aoff + 64],
                                lhsT=at_sb[
                                    0:mlen, mc, h * N + noff : h * N + noff + nlen
                                ],
                                rhs=vb[0:mlen, mc, h * 65 : h * 65 + 64],
                                start=(mc == 0),
                                stop=(mc == 1),
                            )
                    if hblk == 3:
                        nc.scalar.copy(
                            ob[0:nlen, nc_i, 0:512], av[0:nlen, 0:512]
                        )
                        nc.scalar.copy(
                            ob[0:nlen, nc_i, 512:768], av[0:nlen, 512:768]
                        )

                for mc in range(2):
                    ptile = ptp.tile([128, H * N], BF16, tag="pt")
                    pt_t[(b, mc)] = ptile
                order = [
                    ("st", 0, 0), ("av", 0, 0), ("st", 0, 1), ("av", 0, 1),
                    ("st", 0, 2), ("av", 0, 2), ("st", 1, 0), ("av", 0, 3),
                    ("st", 1, 1), ("av", 1, 0), ("st", 1, 2), ("av", 1, 1),
                    ("av", 1, 2), ("av", 1, 3),
                ]
                for kind, a, bb_ in order:
                    if kind == "st":
                        st_unit(a, bb_)
                    else:
                        av_unit(a, bb_)

            def emit_uo(b):
                vb = vb_t[b]
                ob = ob_t[b]
                for nc_i, (noff, nlen) in enumerate(NCH):
                    # two 1-bank halves (6 heads each) so each PSUM slot
                    # frees right after its own half-size mult on DVE
                    rec = small.tile([128, H], F32, tag="rec")
                    tmp = small.tile([128, C], F32, tag="tmp")
                    for half in range(2):
                        uph = psA.tile([128, 512], F32, tag="pA")
                        for h in range(half * 6, half * 6 + 6):
                            uoff = (h % 6) * 65
                            for mc, (moff, mlen) in enumerate(NCH):
                                pt = pt_t[(b, mc)]
                                nc.tensor.matmul(
                                    uph[0:nlen, uoff : uoff + 65],
                                    lhsT=pt[
                                        0:mlen, h * N + noff : h * N + noff + nlen
                                    ],
                                    rhs=vb[0:mlen, mc, h * 65 : h * 65 + 65],
                                    start=(mc == 0),
                                    stop=(mc == 1),
                                )
                        upv = uph[0:nlen, 0:390].rearrange("p (h x) -> p h x", h=6)
                        recv = rec[0:nlen, half * 6 : half * 6 + 6, None]
                        nc.vector.reciprocal(recv, upv[:, :, 64:65])
                        nc.vector.tensor_tensor(
                            tmp[0:nlen, half * 384 : (half + 1) * 384].rearrange(
                                "p (h c) -> p h c", h=6
                            ),
                            upv[:, :, 0:64],
                            recv.to_broadcast((nlen, 6, 64)),
                            AluOp.mult,
                        )
                    nc.vector.tensor_tensor(
                        ob[0:nlen, nc_i, :],
                        tmp[0:nlen, :],
                        ob[0:nlen, nc_i, :],
                        AluOp.add,
                    )

            def emit_tr(b):
                ob = ob_t[b]
                for nc_i, (noff, nlen) in enumerate(NCH):
                    for hp in range(KC):
                        tp = psA.tile([128, 512], BF16, tag="pA")
                        nc.tensor.transpose(
                            tp[:, 0:nlen],
                            in_=ob[0:nlen, nc_i, hp * 128 : (hp + 1) * 128],
                            identity=ident[0:nlen, 0:nlen],
                        )
                        nc.vector.tensor_copy(
                            aot_sb[:, hp, b * N + noff : b * N + noff + nlen],
                            tp[:, 0:nlen],
                        )

            def emit_proj_chunk(mc, pps, tag="pp"):
                moff, mlen = MCS[mc]
                pp = pps.tile([128, 1024], F32, tag=tag)
                for nsl, nw in ((0, 512), (512, 256)):
                    for kc in range(KC):
                        nc.tensor.matmul(
                            pp[0:mlen, nsl : nsl + nw],
                            lhsT=aot_sb[:, kc, moff : moff + mlen],
                            rhs=wproj_sb[:, kc, nsl : nsl + nw],
                            start=(kc == 0),
                            stop=(kc == KC - 1),
                        )
                ot = outst.tile([128, C], F32, tag="ot")
                nc.vector.tensor_tensor(
                    ot[0:mlen, :],
                    pp[0:mlen, 0:768],
                    bias_bc[0:mlen, :],
                    AluOp.add,
                )
                nc.sync.dma_start(
                    out=out_d[moff : moff + mlen, :], in_=ot[0:mlen, :]
                )

            with (
                tc.tile_pool(name="psA", bufs=2, space=MemorySpace.PSUM) as psA,
                tc.tile_pool(name="psB", bufs=3, space=MemorySpace.PSUM) as psB,
            ):
                # input DMAs for batch 0, then weights in use-order
                emit_load_x(0)
                for sec in (0, C):
                    for kc in range(KC):
                        nc.sync.dma_start(
                            out=wqkv_sb[:, kc, sec : sec + C],
                            in_=wqkv_d[kc * 128 : (kc + 1) * 128, sec : sec + C],
                        )
                for kc in range(KC):
                    nc.sync.dma_start(
                        out=wqkv_sb[:, kc, 2 * C : 3 * C],
                        in_=wqkv_d[kc * 128 : (kc + 1) * 128, 2 * C : 3 * C],
                    )
                for mc in range(2):
                    for hh in range(2):
                        nc.sync.dma_start(
                            out=at_sb[:, mc, hh * 6 * N : (hh + 1) * 6 * N],
                            in_=at_d[mc, :, hh * 6 * N : (hh + 1) * 6 * N],
                        )
                stqk = {}
                for b in range(B):
                    if b + 1 < B:
                        emit_load_x(b + 1)
                    stqk[b] = emit_qkT(b)
                    emit_v(b)
                    if b > 0:
                        emit_uo(b - 1)
                        emit_tr(b - 1)
                    emit_st_av(b, *stqk[b])
                    if b == 1:
                        # projection weights stream in behind the early batches
                        for kc in range(KC):
                            nc.sync.dma_start(
                                out=wproj_sb[:, kc, :],
                                in_=wproj_d[kc * 128 : (kc + 1) * 128, :],
                            )
                        bproj_ap = bass.AP(
                            tensor=bproj_d.ap().tensor,
                            offset=0,
                            ap=[[0, 128], [1, C]],
                        )
                        nc.gpsimd.dma_start(out=bias_bc, in_=bproj_ap)
                emit_uo(B - 1)
                emit_tr(B - 1)
                # projection inside the same PSUM scope (no pool-transition
                # stall); early chunks only depend on early batches
                for mc in range(len(MCS)):
                    emit_proj_chunk(mc, psB, tag="pB")

    return nc


_CACHE: dict = {}


def _get_module():
    if "nc" not in _CACHE:
        nc = bacc.Bacc(None, target_bir_lowering=False)
        _emit(nc)
        nc.compile()
        _CACHE["nc"] = nc
    return _CACHE["nc"]


_last_results = None


def kernel(x, W_qkv, static_a, W_proj, b_proj):
    global _last_results
    bf = ml_dtypes.bfloat16
    x = np.asarray(x, dtype=np.float32)
    wqkv_b = np.asarray(W_qkv, dtype=np.float32).astype(bf)
    A = np.asarray(static_a, dtype=np.float32)[0]  # [H, N, N]
    Am = np.ascontiguousarray(A.transpose(2, 0, 1))  # [m, H, n]
    at_arr = np.zeros((2, 128, H, N), dtype=np.float32)
    at_arr[0] = Am[0:128]
    at_arr[1, 0:68] = Am[128:196]
    at_b = at_arr.reshape(2, 128, H * N).astype(bf)
    wproj_b = np.asarray(W_proj, dtype=np.float32).astype(bf)
    bproj_b = np.asarray(b_proj, dtype=np.float32).reshape(1, C).astype(bf)

    in_maps = []
    for i in range(N_CORES):
        shard = x[i * B : (i + 1) * B]  # [B, N, C]
        # [B, 128, KC*N]: xt[b, p, kc*N + n] = x[b, n, kc*128 + p]
        xt_b = np.ascontiguousarray(
            shard.transpose(0, 2, 1)
            .reshape(B, KC, 128, N)
            .transpose(0, 2, 1, 3)
            .reshape(B, 128, KC * N)
        ).astype(bf)
        in_maps.append(
            dict(xt=xt_b, wqkv=wqkv_b, at=at_b, wproj=wproj_b, bproj=bproj_b)
        )

    nc = _get_module()
    res = run_bass_kernel_spmd(nc, in_maps, core_ids=list(range(N_CORES)))
    _last_results = res
    out = np.concatenate(
        [np.asarray(r["out"]).reshape(B, N, C) for r in res.results], axis=0
    )
    return out.astype(np.float32)
